# revision 21
# baseline (speedup 1.0000x reference)
"""Poisson composition layer on 8 Trainium2 NeuronCores via Bass/Tile.

Math: the reference's FFT pipeline on even/odd symmetric extensions reduces
exactly to real DCT-II/DST-II matrix sandwiches (512x512 constant matrices):

    gx = t @ G^T, gy = G @ t          (spectral gradient; G = S2^T diag(-2*pi*k/W^2) C2)
    gx/gy composited with mask        (elementwise)
    A = C2 @ gx @ S2^T                (DCT-y x DST-x of gx)
    B = S2 @ gy @ C2^T                (DST-y x DCT-x of gy)
    Q = WA*A + WB*B                   (Poisson inverse + IDCT normalization folded in)
    u = C2^T @ Q @ C2                 (inverse DCT-II both axes)
    out = u + (mean1[0]-mean2[0])     (host, O(HW))

Each (b,c) of the 2x3 batch is an independent 10-matmul (512^3) pipeline; one
pair per NeuronCore (cores 6,7 duplicate work, outputs ignored).

Every matmul is emitted as out = lhsT.T @ rhs with the *data* as the stationary
operand and constants as the moving operand, which makes every product come out
in natural layout with zero on-chip transposes:
    step1 = matmul(lhsT=X,    rhs=M1T) = (M1 @ X)^T
    step2 = matmul(lhsT=step1, rhs=M2) = M1 @ X @ M2

Matmuls run in float32r (full PE rate; ~13 effective mantissa bits) with fp32
PSUM accumulation; end-to-end error vs the fp32 reference is ~4e-4 relative.

Scheduling notes:
- PSUM evictions and elementwise work are emitted per 128-row block and spread
  across Vector/Scalar/GpSimd so the PE stream stays the critical path.
- The Q-combine reads A and B directly from PSUM (no eviction).
- Host passes every 512x512 operand pre-shuffled to the SBUF tile layout
  [128, 4*512] so each load is one large contiguous-per-partition DMA, ordered
  so the first matmuls' operands arrive first.
"""

import math
import sys

import numpy as np

for _p in ("/opt/trn_rl_repo", "/root/.axon_site/_ro/trn_rl_repo"):
    if _p not in sys.path:
        sys.path.append(_p)

_H = 512
_W = 512
_P = 128
_NB = _W // _P  # 4 partition blocks per 512x512 matrix

_MM_MODE = "f32r"


def _to_tile_layout(m):
    # [512, 512] -> [128, 2048]: block ki (rows 128ki..128ki+127) at cols 512ki..
    m = np.ascontiguousarray(m, dtype=np.float32)
    return np.ascontiguousarray(
        m.reshape(_NB, _P, _W).transpose(1, 0, 2).reshape(_P, _NB * _W)
    )


def _from_tile_layout(t):
    return np.ascontiguousarray(
        t.reshape(_P, _NB, _W).transpose(1, 0, 2).reshape(_H, _W)
    )


def _build_constants():
    W, H = _W, _H
    k = np.arange(W, dtype=np.float64)
    j = np.arange(W, dtype=np.float64)
    ang = np.pi * (2.0 * j[None, :] + 1.0) * k[:, None] / (2.0 * W)
    C2 = np.cos(ang)
    S2 = np.sin(ang)
    G = (S2.T * (-2.0 * np.pi * k / W**2)[None, :]) @ C2

    ky = np.arange(H, dtype=np.float64)[:, None]
    kx = np.arange(W, dtype=np.float64)[None, :]
    dden = 1e-10 - np.pi**2 * (kx**2 / W**2 + ky**2 / H**2)
    cy = np.ones((H, 1)); cy[0, 0] = 0.5
    cx = np.ones((1, W)); cx[0, 0] = 0.5
    WA = (4.0 * np.pi * kx * cy) / (H * W * W * dden)
    WA[:, 0] = 0.0
    WB = (4.0 * np.pi * ky * cx) / (H * W * H * dden)
    WB[0, :] = 0.0

    return {
        "cGT": _to_tile_layout(G.T),
        "cC2T": _to_tile_layout(C2.T),
        "cS2T": _to_tile_layout(S2.T),
        "cC2": _to_tile_layout(C2),
        "cWA": _to_tile_layout(WA),
        "cWB": _to_tile_layout(WB),
    }


_CONSTS = None


def _consts():
    global _CONSTS
    if _CONSTS is None:
        _CONSTS = _build_constants()
    return _CONSTS


# Load order = first-use order, so early matmuls' operands arrive first.
_IN_NAMES = ["tT", "cGT", "sT", "t", "s", "m", "im", "cC2T", "cS2T", "cC2", "cWA", "cWB"]
# Tensors consumed by matmuls (must be float32r end-to-end).
_MM_FED = {"tT", "cGT", "sT", "t", "s", "cC2T", "cS2T", "cC2"}


def _build_program():
    import concourse.bacc as bacc
    import concourse.mybir as mybir
    import concourse.tile as tile

    f32 = mybir.dt.float32
    use_f32r = _MM_MODE == "f32r"
    mm_dt = mybir.dt.float32r if use_f32r else f32

    # Bacc (not raw Bass): its compile() pipeline runs
    # move_matmul_waits_to_ldweights + generate_event_semaphores, which split
    # multi-semaphore waits down to the 1-wait-per-instruction TRN2 limit.
    nc = bacc.Bacc(None, target_bir_lowering=False, debug=False)

    TL = [_P, _NB * _W]

    def in_dt(name):
        return mm_dt if name in _MM_FED else f32

    dram = {
        n: nc.dram_tensor(n, TL, in_dt(n), kind="ExternalInput") for n in _IN_NAMES
    }
    dram_u = nc.dram_tensor("u", TL, f32, kind="ExternalOutput")

    with tile.TileContext(nc) as tc:
        with (
            tc.tile_pool(name="persist", bufs=1) as perpool,
            tc.tile_pool(name="work", bufs=9) as wpool,
            tc.tile_pool(name="psum", bufs=8, space="PSUM") as ppool,
        ):
            # Whole-tensor DMAs (8KB packets per partition run — splitting
            # into column blocks quarters the packet size and the per-queue
            # bandwidth with it). Issue order = first-use order.
            tiles = {}
            for n in _IN_NAMES:
                t_ = perpool.tile(TL, in_dt(n), tag=n)
                nc.sync.dma_start(out=t_[:], in_=dram[n][:])
                tiles[n] = t_

            def blk(t_, mj):
                return t_[:, mj * _W : (mj + 1) * _W]

            def mm512_psum(lhsT, rhs):
                """out = lhsT.T @ rhs; returns the 4 PSUM group tiles."""
                groups = []
                for mj in range(_NB):
                    ps = ppool.tile([_P, _W], f32, tag="ps")
                    for ki in range(_NB):
                        nc.tensor.matmul(
                            ps[:],
                            lhsT[:, ki * _W + mj * _P : ki * _W + (mj + 1) * _P],
                            rhs[:, ki * _W : (ki + 1) * _W],
                            start=(ki == 0),
                            stop=(ki == _NB - 1),
                        )
                    groups.append(ps)
                return groups

            def evict(groups, out_dt, engines, tag="work"):
                """Copy PSUM groups to one SBUF tile; engines[mj] in {'v','s'}."""
                out_t = wpool.tile(TL, out_dt, tag=tag)
                for mj, ps in enumerate(groups):
                    if engines[mj] == "v":
                        nc.vector.tensor_copy(blk(out_t, mj), ps[:])
                    else:
                        nc.scalar.copy(blk(out_t, mj), ps[:])
                return out_t

            def mm_group(ps, lhsT, rhs, mj):
                for ki in range(_NB):
                    nc.tensor.matmul(
                        ps[:],
                        lhsT[:, ki * _W + mj * _P : ki * _W + (mj + 1) * _P],
                        rhs[:, ki * _W : (ki + 1) * _W],
                        start=(ki == 0),
                        stop=(ki == _NB - 1),
                    )

            def mm512(lhsT, rhs, out_dt, engines="vvvs", tag="work"):
                return evict(mm512_psum(lhsT, rhs), out_dt, engines, tag=tag)

            def mm512_pair(a_ops, b_ops, tag="work"):
                """Two independent 512^3 matmuls with interleaved PSUM groups
                so each group's LDWEIGHTS prefetches under the other's MMs."""
                (lhsT_a, rhs_a, dt_a, eng_a), (lhsT_b, rhs_b, dt_b, eng_b) = a_ops, b_ops
                out_a = wpool.tile(TL, dt_a, tag=tag)
                out_b = wpool.tile(TL, dt_b, tag=tag)
                for mj in range(_NB):
                    ps_a = ppool.tile([_P, _W], f32, tag="ps")
                    mm_group(ps_a, lhsT_a, rhs_a, mj)
                    ps_b = ppool.tile([_P, _W], f32, tag="ps")
                    mm_group(ps_b, lhsT_b, rhs_b, mj)
                    if eng_a[mj] == "v":
                        nc.vector.tensor_copy(blk(out_a, mj), ps_a[:])
                    else:
                        nc.scalar.copy(blk(out_a, mj), ps_a[:])
                    if eng_b[mj] == "v":
                        nc.vector.tensor_copy(blk(out_b, mj), ps_b[:])
                    else:
                        nc.scalar.copy(blk(out_b, mj), ps_b[:])
                return out_a, out_b

            V, GP = nc.vector, nc.gpsimd

            def composite(base, other, out_dt):
                # out = im*base + m*other, per block, spread across DVE/GpSimd
                q1 = wpool.tile(TL, f32, tag="work")
                q2 = wpool.tile(TL, f32, tag="work")
                o = wpool.tile(TL, out_dt, tag="work")
                q1e = [GP, GP, V, V]
                q2e = [V, V, V, GP]
                ade = [V, GP, V, GP]
                for mj in range(_NB):
                    q1e[mj].tensor_mul(blk(q1, mj), blk(base, mj), blk(tiles["im"], mj))
                    q2e[mj].tensor_mul(blk(q2, mj), blk(other, mj), blk(tiles["m"], mj))
                    ade[mj].tensor_add(blk(o, mj), blk(q1, mj), blk(q2, mj))
                return o

            # Stage 1: spectral gradients (natural layout). gxt/gxs evict on
            # DVE/ACT in parallel (composite-x is latency-critical); gyt/gys
            # on ACT (their composite has plenty of PE cover).
            gxt, gxs = mm512_pair(
                (tiles["tT"], tiles["cGT"], f32, "vvvv"),  # t @ G^T
                (tiles["sT"], tiles["cGT"], f32, "ssss"),  # s @ G^T
            )
            gx = composite(gxt, gxs, mm_dt)
            gyt, gys = mm512_pair(
                (tiles["cGT"], tiles["t"], f32, "ssss"),   # G @ t
                (tiles["cGT"], tiles["s"], f32, "ssss"),   # G @ s
            )
            gy = composite(gyt, gys, mm_dt)

            # Stage 2: forward transforms; p1 first, then A interleaved with
            # the independent p2; A/B evictions on ACT feed the DVE Q-combine
            p1 = mm512(gx, tiles["cC2T"], mm_dt, engines="vvvs")  # (C2 gx)^T
            A, p2 = mm512_pair(
                (p1, tiles["cS2T"], f32, "ssss"),          # C2 gx S2^T
                (gy, tiles["cS2T"], mm_dt, "vvvv"),        # (S2 gy)^T
            )
            Bm = mm512(p2, tiles["cC2T"], f32, engines="ssss")    # S2 gy C2^T

            # Stage 3: Q = WA*A + WB*B per block; qa early (overlaps p2/B),
            # post-B chain per block on alternating engines
            qa = wpool.tile(TL, f32, tag="work")
            qb = wpool.tile(TL, f32, tag="work")
            q = wpool.tile(TL, mm_dt, tag="work")
            for mj in range(_NB):
                V.tensor_mul(blk(qa, mj), blk(A, mj), blk(tiles["cWA"], mj))
            qbe = [V, GP, V, GP]
            for mj in range(_NB):
                qbe[mj].tensor_mul(blk(qb, mj), blk(Bm, mj), blk(tiles["cWB"], mj))
                qbe[mj].tensor_add(blk(q, mj), blk(qa, mj), blk(qb, mj))

            # Stage 4: inverse DCT both axes; stream result out per block
            p3 = mm512(q, tiles["cC2"], mm_dt, engines="vvvs")    # (C2^T Q)^T
            u_ps = mm512_psum(p3, tiles["cC2"])                   # C2^T Q C2
            u = wpool.tile(TL, f32, tag="work")
            for mj in range(_NB):
                eng = nc.vector if mj % 2 == 0 else nc.scalar
                if mj % 2 == 0:
                    nc.vector.tensor_copy(blk(u, mj), u_ps[mj][:])
                else:
                    nc.scalar.copy(blk(u, mj), u_ps[mj][:])
                nc.sync.dma_start(
                    out=dram_u[:, mj * _W : (mj + 1) * _W], in_=blk(u, mj)
                )

    nc.compile()
    return nc


def _run_device(t_all, s_all, m_all, trace=False):
    from concourse.bass_utils import run_bass_kernel_spmd

    B, C, H, W = t_all.shape
    consts = _consts()
    nc = _build_program()

    pairs = [(b, c) for b in range(B) for c in range(C)]
    n_cores = 8
    core_ids = list(range(n_cores))
    in_maps = []
    for i in range(n_cores):
        b, c = pairs[i % len(pairs)]
        t = t_all[b, c]
        s = s_all[b, c]
        m = m_all[b, c]
        im = {
            "t": _to_tile_layout(t),
            "tT": _to_tile_layout(t.T),
            "s": _to_tile_layout(s),
            "sT": _to_tile_layout(s.T),
            "m": _to_tile_layout(m),
            "im": _to_tile_layout(1.0 - m),
        }
        im.update(consts)
        in_maps.append(im)

    r = run_bass_kernel_spmd(nc, in_maps, core_ids, trace=trace)
    res = np.zeros((B, C, H, W), dtype=np.float32)
    for i, (b, c) in enumerate(pairs):
        res[b, c] = _from_tile_layout(r.results[i]["u"])
    return res, r


def kernel(target, source, mask):
    t_all = np.asarray(target, dtype=np.float32)
    s_all = np.asarray(source, dtype=np.float32)
    m_all = np.asarray(mask, dtype=np.float32)

    res, _ = _run_device(t_all, s_all, m_all, trace=False)

    # Mean-matching on the unmasked region; batch 0's means applied to all
    # batches (exactly as the reference does). O(HW) -> host.
    inv = 1.0 - m_all.astype(np.float64)
    t64 = t_all.astype(np.float64)
    r64 = res.astype(np.float64)
    denom = inv.sum(axis=(-1, -2))
    mean1 = (t64 * inv).sum(axis=(-1, -2)) / denom
    mean2 = (r64 * inv).sum(axis=(-1, -2)) / denom
    out = r64 + (mean1[0] - mean2[0])[None, :, None, None]
    return out.astype(np.float32)


# revision 22
# speedup vs baseline: 1.2585x; 1.2585x over previous
"""Poisson composition layer on 8 Trainium2 NeuronCores via Bass/Tile.

Math: the reference's FFT pipeline on even/odd symmetric extensions reduces
exactly to real DCT-II/DST-II matrix sandwiches (512x512 constant matrices):

    gx = t @ G^T, gy = G @ t          (spectral gradient; G = S2^T diag(-2*pi*k/W^2) C2)
    gx/gy composited with mask        (elementwise)
    A = C2 @ gx @ S2^T                (DCT-y x DST-x of gx)
    B = S2 @ gy @ C2^T                (DST-y x DCT-x of gy)
    Q = WA*A + WB*B                   (Poisson inverse + IDCT normalization folded in)
    u = C2^T @ Q @ C2                 (inverse DCT-II both axes)
    out = u + (mean1[0]-mean2[0])     (host, O(HW))

Each (b,c) of the 2x3 batch is an independent 10-matmul (512^3) pipeline; one
pair per NeuronCore (cores 6,7 duplicate work, outputs ignored).

Every matmul is emitted as out = lhsT.T @ rhs with the *data* as the stationary
operand and constants as the moving operand, which makes every product come out
in natural layout with zero on-chip transposes:
    step1 = matmul(lhsT=X,    rhs=M1T) = (M1 @ X)^T
    step2 = matmul(lhsT=step1, rhs=M2) = M1 @ X @ M2

Matmuls run in float32r (full PE rate; ~13 effective mantissa bits) with fp32
PSUM accumulation; end-to-end error vs the fp32 reference is ~4e-4 relative.

Scheduling notes:
- PSUM evictions and elementwise work are emitted per 128-row block and spread
  across Vector/Scalar/GpSimd so the PE stream stays the critical path.
- The Q-combine reads A and B directly from PSUM (no eviction).
- Host passes every 512x512 operand pre-shuffled to the SBUF tile layout
  [128, 4*512] so each load is one large contiguous-per-partition DMA, ordered
  so the first matmuls' operands arrive first.
"""

import math
import sys

import numpy as np

for _p in ("/opt/trn_rl_repo", "/root/.axon_site/_ro/trn_rl_repo"):
    if _p not in sys.path:
        sys.path.append(_p)

_H = 512
_W = 512
_P = 128
_NB = _W // _P  # 4 partition blocks per 512x512 matrix

_MM_MODE = "f32r"


def _to_tile_layout(m):
    # [512, 512] -> [128, 2048]: block ki (rows 128ki..128ki+127) at cols 512ki..
    m = np.ascontiguousarray(m, dtype=np.float32)
    return np.ascontiguousarray(
        m.reshape(_NB, _P, _W).transpose(1, 0, 2).reshape(_P, _NB * _W)
    )


def _from_tile_layout(t):
    return np.ascontiguousarray(
        t.reshape(_P, _NB, _W).transpose(1, 0, 2).reshape(_H, _W)
    )


def _build_constants():
    W, H = _W, _H
    k = np.arange(W, dtype=np.float64)
    j = np.arange(W, dtype=np.float64)
    ang = np.pi * (2.0 * j[None, :] + 1.0) * k[:, None] / (2.0 * W)
    C2 = np.cos(ang)
    S2 = np.sin(ang)
    G = (S2.T * (-2.0 * np.pi * k / W**2)[None, :]) @ C2

    ky = np.arange(H, dtype=np.float64)[:, None]
    kx = np.arange(W, dtype=np.float64)[None, :]
    dden = 1e-10 - np.pi**2 * (kx**2 / W**2 + ky**2 / H**2)
    cy = np.ones((H, 1)); cy[0, 0] = 0.5
    cx = np.ones((1, W)); cx[0, 0] = 0.5
    WA = (4.0 * np.pi * kx * cy) / (H * W * W * dden)
    WA[:, 0] = 0.0
    WB = (4.0 * np.pi * ky * cx) / (H * W * H * dden)
    WB[0, :] = 0.0

    return {
        "cGT": _to_tile_layout(G.T),
        "cC2T": _to_tile_layout(C2.T),
        "cS2T": _to_tile_layout(S2.T),
        "cC2": _to_tile_layout(C2),
        "cWA": _to_tile_layout(WA),
        "cWB": _to_tile_layout(WB),
    }


_CONSTS = None


def _consts():
    global _CONSTS
    if _CONSTS is None:
        _CONSTS = _build_constants()
    return _CONSTS


# Load order = first-use order, so early matmuls' operands arrive first.
_IN_NAMES = ["tT", "cGT", "sT", "t", "s", "m", "im", "cC2T", "cS2T", "cC2", "cWA", "cWB"]
# Tensors consumed by matmuls (must be float32r end-to-end).
_MM_FED = {"tT", "cGT", "sT", "t", "s", "cC2T", "cS2T", "cC2"}


def _build_program():
    import concourse.bacc as bacc
    import concourse.mybir as mybir
    import concourse.tile as tile

    f32 = mybir.dt.float32
    use_f32r = _MM_MODE == "f32r"
    mm_dt = mybir.dt.float32r if use_f32r else f32

    # Bacc (not raw Bass): its compile() pipeline runs
    # move_matmul_waits_to_ldweights + generate_event_semaphores, which split
    # multi-semaphore waits down to the 1-wait-per-instruction TRN2 limit.
    nc = bacc.Bacc(None, target_bir_lowering=False, debug=False)

    TL = [_P, _NB * _W]

    def in_dt(name):
        return mm_dt if name in _MM_FED else f32

    dram = {
        n: nc.dram_tensor(n, TL, in_dt(n), kind="ExternalInput") for n in _IN_NAMES
    }
    dram_u = nc.dram_tensor("u", TL, f32, kind="ExternalOutput")

    with tile.TileContext(nc) as tc:
        with (
            tc.tile_pool(name="persist", bufs=1) as perpool,
            tc.tile_pool(name="work", bufs=9) as wpool,
            tc.tile_pool(name="psum", bufs=8, space="PSUM") as ppool,
        ):
            # Whole-tensor DMAs (8KB packets per partition run — splitting
            # into column blocks quarters the packet size and the per-queue
            # bandwidth with it). Issue order = first-use order.
            tiles = {}
            for n in _IN_NAMES:
                t_ = perpool.tile(TL, in_dt(n), tag=n)
                nc.sync.dma_start(out=t_[:], in_=dram[n][:])
                tiles[n] = t_

            def blk(t_, mj):
                return t_[:, mj * _W : (mj + 1) * _W]

            def mm512_psum(lhsT, rhs):
                """out = lhsT.T @ rhs; returns the 4 PSUM group tiles."""
                groups = []
                for mj in range(_NB):
                    ps = ppool.tile([_P, _W], f32, tag="ps")
                    for ki in range(_NB):
                        nc.tensor.matmul(
                            ps[:],
                            lhsT[:, ki * _W + mj * _P : ki * _W + (mj + 1) * _P],
                            rhs[:, ki * _W : (ki + 1) * _W],
                            start=(ki == 0),
                            stop=(ki == _NB - 1),
                        )
                    groups.append(ps)
                return groups

            def evict(groups, out_dt, engines, tag="work"):
                """Copy PSUM groups to one SBUF tile; engines[mj] in {'v','s'}."""
                out_t = wpool.tile(TL, out_dt, tag=tag)
                for mj, ps in enumerate(groups):
                    if engines[mj] == "v":
                        nc.vector.tensor_copy(blk(out_t, mj), ps[:])
                    else:
                        nc.scalar.copy(blk(out_t, mj), ps[:])
                return out_t

            def mm_group(ps, lhsT, rhs, mj):
                for ki in range(_NB):
                    nc.tensor.matmul(
                        ps[:],
                        lhsT[:, ki * _W + mj * _P : ki * _W + (mj + 1) * _P],
                        rhs[:, ki * _W : (ki + 1) * _W],
                        start=(ki == 0),
                        stop=(ki == _NB - 1),
                    )

            def mm512(lhsT, rhs, out_dt, engines="vvvs", tag="work"):
                return evict(mm512_psum(lhsT, rhs), out_dt, engines, tag=tag)

            def mm512_pair(a_ops, b_ops, tag="work"):
                """Two independent 512^3 matmuls with interleaved PSUM groups
                so each group's LDWEIGHTS prefetches under the other's MMs."""
                (lhsT_a, rhs_a, dt_a, eng_a), (lhsT_b, rhs_b, dt_b, eng_b) = a_ops, b_ops
                out_a = wpool.tile(TL, dt_a, tag=tag)
                out_b = wpool.tile(TL, dt_b, tag=tag)
                for mj in range(_NB):
                    ps_a = ppool.tile([_P, _W], f32, tag="ps")
                    mm_group(ps_a, lhsT_a, rhs_a, mj)
                    ps_b = ppool.tile([_P, _W], f32, tag="ps")
                    mm_group(ps_b, lhsT_b, rhs_b, mj)
                    if eng_a[mj] == "v":
                        nc.vector.tensor_copy(blk(out_a, mj), ps_a[:])
                    else:
                        nc.scalar.copy(blk(out_a, mj), ps_a[:])
                    if eng_b[mj] == "v":
                        nc.vector.tensor_copy(blk(out_b, mj), ps_b[:])
                    else:
                        nc.scalar.copy(blk(out_b, mj), ps_b[:])
                return out_a, out_b

            V, GP = nc.vector, nc.gpsimd

            def composite(base, other, out_dt):
                # out = im*base + m*other, per block, spread across DVE/GpSimd
                q1 = wpool.tile(TL, f32, tag="work")
                q2 = wpool.tile(TL, f32, tag="work")
                o = wpool.tile(TL, out_dt, tag="work")
                q1e = [GP, GP, V, V]
                q2e = [V, V, V, GP]
                ade = [V, GP, V, GP]
                for mj in range(_NB):
                    q1e[mj].tensor_mul(blk(q1, mj), blk(base, mj), blk(tiles["im"], mj))
                    q2e[mj].tensor_mul(blk(q2, mj), blk(other, mj), blk(tiles["m"], mj))
                    ade[mj].tensor_add(blk(o, mj), blk(q1, mj), blk(q2, mj))
                return o

            # Stage 1: spectral gradients (natural layout); gxt on DVE and gxs
            # on ACT so both evict in parallel ahead of the composite
            gxt = mm512(tiles["tT"], tiles["cGT"], f32, engines="vvvv")  # t @ G^T
            gxs = mm512(tiles["sT"], tiles["cGT"], f32, engines="ssss")  # s @ G^T
            gx = composite(gxt, gxs, mm_dt)
            gyt = mm512(tiles["cGT"], tiles["t"], f32, engines="ssss")   # G @ t
            gys = mm512(tiles["cGT"], tiles["s"], f32, engines="ssss")   # G @ s
            gy = composite(gyt, gys, mm_dt)

            # Stage 2: forward transforms; p1/p2 evictions mostly DVE
            # (PE-critical), A/B on ACT (feed the DVE Q-combine)
            p1 = mm512(gx, tiles["cC2T"], mm_dt, engines="vvvs")  # (C2 gx)^T
            A = mm512(p1, tiles["cS2T"], f32, engines="ssss")     # C2 gx S2^T
            p2 = mm512(gy, tiles["cS2T"], mm_dt, engines="vvvs")  # (S2 gy)^T
            Bm = mm512(p2, tiles["cC2T"], f32, engines="ssss")    # S2 gy C2^T

            # Stage 3: Q = WA*A + WB*B per block; qa early (overlaps p2/B),
            # post-B chain per block on alternating engines
            qa = wpool.tile(TL, f32, tag="work")
            qb = wpool.tile(TL, f32, tag="work")
            q = wpool.tile(TL, mm_dt, tag="work")
            for mj in range(_NB):
                V.tensor_mul(blk(qa, mj), blk(A, mj), blk(tiles["cWA"], mj))
            qbe = [V, GP, V, GP]
            for mj in range(_NB):
                qbe[mj].tensor_mul(blk(qb, mj), blk(Bm, mj), blk(tiles["cWB"], mj))
                qbe[mj].tensor_add(blk(q, mj), blk(qa, mj), blk(qb, mj))

            # Stage 4: inverse DCT both axes; stream result out per block
            p3 = mm512(q, tiles["cC2"], mm_dt, engines="vvvs")    # (C2^T Q)^T
            u_ps = mm512_psum(p3, tiles["cC2"])                   # C2^T Q C2
            u = wpool.tile(TL, f32, tag="work")
            for mj in range(_NB):
                eng = nc.vector if mj % 2 == 0 else nc.scalar
                if mj % 2 == 0:
                    nc.vector.tensor_copy(blk(u, mj), u_ps[mj][:])
                else:
                    nc.scalar.copy(blk(u, mj), u_ps[mj][:])
                nc.sync.dma_start(
                    out=dram_u[:, mj * _W : (mj + 1) * _W], in_=blk(u, mj)
                )

    nc.compile()
    return nc


def _run_device(t_all, s_all, m_all, trace=False):
    from concourse.bass_utils import run_bass_kernel_spmd

    B, C, H, W = t_all.shape
    consts = _consts()
    nc = _build_program()

    pairs = [(b, c) for b in range(B) for c in range(C)]
    n_cores = 8
    core_ids = list(range(n_cores))
    in_maps = []
    for i in range(n_cores):
        b, c = pairs[i % len(pairs)]
        t = t_all[b, c]
        s = s_all[b, c]
        m = m_all[b, c]
        im = {
            "t": _to_tile_layout(t),
            "tT": _to_tile_layout(t.T),
            "s": _to_tile_layout(s),
            "sT": _to_tile_layout(s.T),
            "m": _to_tile_layout(m),
            "im": _to_tile_layout(1.0 - m),
        }
        im.update(consts)
        in_maps.append(im)

    r = run_bass_kernel_spmd(nc, in_maps, core_ids, trace=trace)
    res = np.zeros((B, C, H, W), dtype=np.float32)
    for i, (b, c) in enumerate(pairs):
        res[b, c] = _from_tile_layout(r.results[i]["u"])
    return res, r


def kernel(target, source, mask):
    t_all = np.asarray(target, dtype=np.float32)
    s_all = np.asarray(source, dtype=np.float32)
    m_all = np.asarray(mask, dtype=np.float32)

    res, _ = _run_device(t_all, s_all, m_all, trace=False)

    # Mean-matching on the unmasked region; batch 0's means applied to all
    # batches (exactly as the reference does). O(HW) -> host.
    inv = 1.0 - m_all.astype(np.float64)
    t64 = t_all.astype(np.float64)
    r64 = res.astype(np.float64)
    denom = inv.sum(axis=(-1, -2))
    mean1 = (t64 * inv).sum(axis=(-1, -2)) / denom
    mean2 = (r64 * inv).sum(axis=(-1, -2)) / denom
    out = r64 + (mean1[0] - mean2[0])[None, :, None, None]
    return out.astype(np.float32)


# revision 25
# speedup vs baseline: 1.2882x; 1.0236x over previous
"""Poisson composition layer on 8 Trainium2 NeuronCores via Bass/Tile.

Math: the reference's FFT pipeline on even/odd symmetric extensions reduces
exactly to real DCT-II/DST-II matrix sandwiches (512x512 constant matrices):

    gx = t @ G^T, gy = G @ t          (spectral gradient; G = S2^T diag(-2*pi*k/W^2) C2)
    gx/gy composited with mask        (elementwise)
    A = C2 @ gx @ S2^T                (DCT-y x DST-x of gx)
    B = S2 @ gy @ C2^T                (DST-y x DCT-x of gy)
    Q = WA*A + WB*B                   (Poisson inverse + IDCT normalization folded in)
    u = C2^T @ Q @ C2                 (inverse DCT-II both axes)
    out = u + (mean1[0]-mean2[0])     (host, O(HW))

Each (b,c) of the 2x3 batch is an independent 10-matmul (512^3) pipeline; one
pair per NeuronCore (cores 6,7 duplicate work, outputs ignored).

Every matmul is emitted as out = lhsT.T @ rhs with the *data* as the stationary
operand and constants as the moving operand, which makes every product come out
in natural layout with zero on-chip transposes:
    step1 = matmul(lhsT=X,    rhs=M1T) = (M1 @ X)^T
    step2 = matmul(lhsT=step1, rhs=M2) = M1 @ X @ M2

Matmuls run in float32r (full PE rate; ~13 effective mantissa bits) with fp32
PSUM accumulation; end-to-end error vs the fp32 reference is ~4e-4 relative.

Scheduling notes:
- PSUM evictions and elementwise work are emitted per 128-row block and spread
  across Vector/Scalar/GpSimd so the PE stream stays the critical path.
- The Q-combine reads A and B directly from PSUM (no eviction).
- Host passes every 512x512 operand pre-shuffled to the SBUF tile layout
  [128, 4*512] so each load is one large contiguous-per-partition DMA, ordered
  so the first matmuls' operands arrive first.
"""

import math
import sys

import numpy as np

for _p in ("/opt/trn_rl_repo", "/root/.axon_site/_ro/trn_rl_repo"):
    if _p not in sys.path:
        sys.path.append(_p)

_H = 512
_W = 512
_P = 128
_NB = _W // _P  # 4 partition blocks per 512x512 matrix

_MM_MODE = "f32r"


def _to_tile_layout(m):
    # [512, 512] -> [128, 2048]: block ki (rows 128ki..128ki+127) at cols 512ki..
    m = np.ascontiguousarray(m, dtype=np.float32)
    return np.ascontiguousarray(
        m.reshape(_NB, _P, _W).transpose(1, 0, 2).reshape(_P, _NB * _W)
    )


def _from_tile_layout(t):
    return np.ascontiguousarray(
        t.reshape(_P, _NB, _W).transpose(1, 0, 2).reshape(_H, _W)
    )


def _build_constants():
    W, H = _W, _H
    k = np.arange(W, dtype=np.float64)
    j = np.arange(W, dtype=np.float64)
    ang = np.pi * (2.0 * j[None, :] + 1.0) * k[:, None] / (2.0 * W)
    C2 = np.cos(ang)
    S2 = np.sin(ang)
    G = (S2.T * (-2.0 * np.pi * k / W**2)[None, :]) @ C2

    ky = np.arange(H, dtype=np.float64)[:, None]
    kx = np.arange(W, dtype=np.float64)[None, :]
    dden = 1e-10 - np.pi**2 * (kx**2 / W**2 + ky**2 / H**2)
    cy = np.ones((H, 1)); cy[0, 0] = 0.5
    cx = np.ones((1, W)); cx[0, 0] = 0.5
    WA = (4.0 * np.pi * kx * cy) / (H * W * W * dden)
    WA[:, 0] = 0.0
    WB = (4.0 * np.pi * ky * cx) / (H * W * H * dden)
    WB[0, :] = 0.0

    return {
        "cGT": _to_tile_layout(G.T),
        "cC2T": _to_tile_layout(C2.T),
        "cS2T": _to_tile_layout(S2.T),
        "cC2": _to_tile_layout(C2),
        "cWA": _to_tile_layout(WA),
        "cWB": _to_tile_layout(WB),
    }


_CONSTS = None


def _consts():
    global _CONSTS
    if _CONSTS is None:
        _CONSTS = _build_constants()
    return _CONSTS


# Load order = first-use order, so early matmuls' operands arrive first.
_IN_NAMES = ["tT", "cGT", "sT", "t", "s", "m", "im", "cC2T", "cS2T", "cC2", "cWA", "cWB"]
# Tensors consumed by matmuls (must be float32r end-to-end).
_MM_FED = {"tT", "cGT", "sT", "t", "s", "cC2T", "cS2T", "cC2"}


def _build_program():
    import concourse.bacc as bacc
    import concourse.mybir as mybir
    import concourse.tile as tile

    f32 = mybir.dt.float32
    use_f32r = _MM_MODE == "f32r"
    mm_dt = mybir.dt.float32r if use_f32r else f32

    # Bacc (not raw Bass): its compile() pipeline runs
    # move_matmul_waits_to_ldweights + generate_event_semaphores, which split
    # multi-semaphore waits down to the 1-wait-per-instruction TRN2 limit.
    nc = bacc.Bacc(None, target_bir_lowering=False, debug=False)

    TL = [_P, _NB * _W]

    def in_dt(name):
        return mm_dt if name in _MM_FED else f32

    dram = {
        n: nc.dram_tensor(n, TL, in_dt(n), kind="ExternalInput") for n in _IN_NAMES
    }
    dram_u = nc.dram_tensor("u", TL, f32, kind="ExternalOutput")

    with tile.TileContext(nc) as tc:
        with (
            tc.tile_pool(name="persist", bufs=1) as perpool,
            tc.tile_pool(name="work", bufs=9) as wpool,
            tc.tile_pool(name="psum", bufs=8, space="PSUM") as ppool,
        ):
            # Whole-tensor DMAs (8KB packets per partition run — splitting
            # into column blocks quarters the packet size and the per-queue
            # bandwidth with it). Issue order = first-use order.
            tiles = {}
            for n in _IN_NAMES:
                t_ = perpool.tile(TL, in_dt(n), tag=n)
                nc.sync.dma_start(out=t_[:], in_=dram[n][:])
                tiles[n] = t_

            def blk(t_, mj):
                return t_[:, mj * _W : (mj + 1) * _W]

            def mm512_psum(lhsT, rhs):
                """out = lhsT.T @ rhs; returns the 4 PSUM group tiles."""
                groups = []
                for mj in range(_NB):
                    ps = ppool.tile([_P, _W], f32, tag="ps")
                    for ki in range(_NB):
                        nc.tensor.matmul(
                            ps[:],
                            lhsT[:, ki * _W + mj * _P : ki * _W + (mj + 1) * _P],
                            rhs[:, ki * _W : (ki + 1) * _W],
                            start=(ki == 0),
                            stop=(ki == _NB - 1),
                        )
                    groups.append(ps)
                return groups

            def evict(groups, out_dt, engines, tag="work"):
                """Copy PSUM groups to one SBUF tile; engines[mj] in {'v','s'}."""
                out_t = wpool.tile(TL, out_dt, tag=tag)
                for mj, ps in enumerate(groups):
                    if engines[mj] == "v":
                        nc.vector.tensor_copy(blk(out_t, mj), ps[:])
                    else:
                        nc.scalar.copy(blk(out_t, mj), ps[:])
                return out_t

            def mm_group(ps, lhsT, rhs, mj):
                for ki in range(_NB):
                    nc.tensor.matmul(
                        ps[:],
                        lhsT[:, ki * _W + mj * _P : ki * _W + (mj + 1) * _P],
                        rhs[:, ki * _W : (ki + 1) * _W],
                        start=(ki == 0),
                        stop=(ki == _NB - 1),
                    )

            def mm512(lhsT, rhs, out_dt, engines="vvvs", tag="work"):
                return evict(mm512_psum(lhsT, rhs), out_dt, engines, tag=tag)

            def mm512_pair(a_ops, b_ops, tag="work"):
                """Two independent 512^3 matmuls with interleaved PSUM groups
                so each group's LDWEIGHTS prefetches under the other's MMs."""
                (lhsT_a, rhs_a, dt_a, eng_a), (lhsT_b, rhs_b, dt_b, eng_b) = a_ops, b_ops
                out_a = wpool.tile(TL, dt_a, tag=tag)
                out_b = wpool.tile(TL, dt_b, tag=tag)
                for mj in range(_NB):
                    ps_a = ppool.tile([_P, _W], f32, tag="ps")
                    mm_group(ps_a, lhsT_a, rhs_a, mj)
                    ps_b = ppool.tile([_P, _W], f32, tag="ps")
                    mm_group(ps_b, lhsT_b, rhs_b, mj)
                    if eng_a[mj] == "v":
                        nc.vector.tensor_copy(blk(out_a, mj), ps_a[:])
                    else:
                        nc.scalar.copy(blk(out_a, mj), ps_a[:])
                    if eng_b[mj] == "v":
                        nc.vector.tensor_copy(blk(out_b, mj), ps_b[:])
                    else:
                        nc.scalar.copy(blk(out_b, mj), ps_b[:])
                return out_a, out_b

            V, GP = nc.vector, nc.gpsimd

            def composite(base, other, out_dt, q1_eng=None):
                # out = im*base + m*other. Full-width ops: zero-offset APs hit
                # the DVE fast path (~2.6x faster than mid-tile block slices).
                # q1 only needs `base` (available early), so it can ride on
                # GpSimd for the non-critical composite.
                q1e = q1_eng or V
                q1 = wpool.tile(TL, f32, tag="work")
                q2 = wpool.tile(TL, f32, tag="work")
                o = wpool.tile(TL, out_dt, tag="work")
                q1e.tensor_mul(q1[:], base[:], tiles["im"][:])
                V.tensor_mul(q2[:], other[:], tiles["m"][:])
                V.tensor_add(o[:], q1[:], q2[:])
                return o

            # Stage 1: spectral gradients (natural layout); gxt on DVE and gxs
            # on ACT so both evict in parallel ahead of the composite
            gxt = mm512(tiles["tT"], tiles["cGT"], f32, engines="vvvv")  # t @ G^T
            gxs = mm512(tiles["sT"], tiles["cGT"], f32, engines="ssss")  # s @ G^T
            gx = composite(gxt, gxs, mm_dt)
            gyt = mm512(tiles["cGT"], tiles["t"], f32, engines="ssss")   # G @ t
            gys = mm512(tiles["cGT"], tiles["s"], f32, engines="ssss")   # G @ s
            gy = composite(gyt, gys, mm_dt, q1_eng=GP)

            # Stage 2: forward transforms; p1/p2 evictions mostly DVE
            # (PE-critical), A/B on ACT (feed the DVE Q-combine)
            p1 = mm512(gx, tiles["cC2T"], mm_dt, engines="vvvs")  # (C2 gx)^T
            A = mm512(p1, tiles["cS2T"], f32, engines="ssss")     # C2 gx S2^T
            p2 = mm512(gy, tiles["cS2T"], mm_dt, engines="vvvs")  # (S2 gy)^T
            Bm = mm512(p2, tiles["cC2T"], f32, engines="ssss")    # S2 gy C2^T

            # Stage 3: Q = WA*A + WB*B; qa early (overlaps p2/B), qb + add
            # full-width on DVE right after B
            qa = wpool.tile(TL, f32, tag="work")
            qb = wpool.tile(TL, f32, tag="work")
            q = wpool.tile(TL, mm_dt, tag="work")
            V.tensor_mul(qa[:], A[:], tiles["cWA"][:])
            V.tensor_mul(qb[:], Bm[:], tiles["cWB"][:])
            V.tensor_add(q[:], qa[:], qb[:])

            # Stage 4: inverse DCT both axes; stream result out per block
            p3 = mm512(q, tiles["cC2"], mm_dt, engines="vvvs")    # (C2^T Q)^T
            u_ps = mm512_psum(p3, tiles["cC2"])                   # C2^T Q C2
            u = wpool.tile(TL, f32, tag="work")
            for mj in range(_NB):
                eng = nc.vector if mj % 2 == 0 else nc.scalar
                if mj % 2 == 0:
                    nc.vector.tensor_copy(blk(u, mj), u_ps[mj][:])
                else:
                    nc.scalar.copy(blk(u, mj), u_ps[mj][:])
                nc.sync.dma_start(
                    out=dram_u[:, mj * _W : (mj + 1) * _W], in_=blk(u, mj)
                )

    nc.compile()
    return nc


def _run_device(t_all, s_all, m_all, trace=False):
    from concourse.bass_utils import run_bass_kernel_spmd

    B, C, H, W = t_all.shape
    consts = _consts()
    nc = _build_program()

    pairs = [(b, c) for b in range(B) for c in range(C)]
    n_cores = 8
    core_ids = list(range(n_cores))
    in_maps = []
    for i in range(n_cores):
        b, c = pairs[i % len(pairs)]
        t = t_all[b, c]
        s = s_all[b, c]
        m = m_all[b, c]
        im = {
            "t": _to_tile_layout(t),
            "tT": _to_tile_layout(t.T),
            "s": _to_tile_layout(s),
            "sT": _to_tile_layout(s.T),
            "m": _to_tile_layout(m),
            "im": _to_tile_layout(1.0 - m),
        }
        im.update(consts)
        in_maps.append(im)

    r = run_bass_kernel_spmd(nc, in_maps, core_ids, trace=trace)
    res = np.zeros((B, C, H, W), dtype=np.float32)
    for i, (b, c) in enumerate(pairs):
        res[b, c] = _from_tile_layout(r.results[i]["u"])
    return res, r


def kernel(target, source, mask):
    t_all = np.asarray(target, dtype=np.float32)
    s_all = np.asarray(source, dtype=np.float32)
    m_all = np.asarray(mask, dtype=np.float32)

    res, _ = _run_device(t_all, s_all, m_all, trace=False)

    # Mean-matching on the unmasked region; batch 0's means applied to all
    # batches (exactly as the reference does). O(HW) -> host.
    inv = 1.0 - m_all.astype(np.float64)
    t64 = t_all.astype(np.float64)
    r64 = res.astype(np.float64)
    denom = inv.sum(axis=(-1, -2))
    mean1 = (t64 * inv).sum(axis=(-1, -2)) / denom
    mean2 = (r64 * inv).sum(axis=(-1, -2)) / denom
    out = r64 + (mean1[0] - mean2[0])[None, :, None, None]
    return out.astype(np.float32)
